# revision 26
# baseline (speedup 1.0000x reference)
"""Trainium2 Bass kernel for single-head self-attention (B=2, S=4096, D=1024).

reference:
    q = x @ Wq; k = x @ Wk; v = x @ Wv          # [B,S,D]
    energy = einsum('bid,bjd->bij', q, k) * 8.0  # SCALE = sqrt(64)
    attn = softmax(energy, axis=-1)
    out = einsum('bij,bjd->bid', attn, v) @ Wo

Weight folding (associativity): energy = x @ (Wq Wk^T) @ x^T and
out = attn @ (x @ (Wv Wo)), so the host precomputes M = Wq @ Wk^T and
W' = Wv @ Wo once (fp64) and the device only runs TWO projections
(G = x@M, V' = x@W') plus the two S^2-sized attention matmuls.

Two SPMD launches over 8 cores (= 2 batches x 4 query-blocks of 1024):
  phase 1: each core computes G / V' for its own 1024 rows (fp16).
  phase 2: each core computes softmax(G_blk @ x^T * 8) @ V' for its
           1024 queries against the full batch.

Phase-2 speed tricks (vs the plain fp16 version):
  * P @ V' runs in fp8-e4m3 DoubleRow mode (K=256 per instruction,
    0.5 PE-cycles/row): the host splits V' into V8hi + V8lo (hi/lo fp8
    pair, residual ~7.5e-4 relative), P is a single fp8 (softmax rows
    are ~99.2% one-hot; quantizing the off-argmax tail costs ~2e-3).
    Two DR sweeps (hi then lo) accumulate into the same PSUM group, so
    the P@V' matmul costs 2*16 instead of 64 fp16-equivalents per
    i-tile: 131072 PE-cycles total instead of 262144.
  * P^T is produced by the DMA xbar transpose (14ns per 16x128 tile on
    the DMA queue; zero PE cycles) into fp16, then GPSIMD (otherwise
    idle) convert-copies it to the fp8 DR operand. This removes all
    256 PE transposes and their DVE psum->SBUF copies.
  * The logit path (G = x@M and E = G@x^T) stays fp16: logit std is
    ~300 (SCALE multiplies) and fp8 quantization noise there flips
    near-tie argmaxes.  fp16 keeps logit noise ~0.13 -> ~8e-3 rel err.
  * PE p-state warmup matmuls run during the DMA lead-in (HAM reaches
    2.4GHz after ~3us of sustained PE activity).

Queue assignment: bulk loads ride SP in consumption order; the ACT
queue handles E psum->SBUF copies + the softmax exp chain + output
scales; DVE takes the reduces; SP also carries the P DMA-transposes
and y stores (issued in dependency order); GPSIMD does the fp8 casts.
"""

import numpy as np
import ml_dtypes

B, S, D = 2, 4096, 1024
BLK = 1024          # queries per core
SCALE = 8.0         # HEAD_DIM ** 0.5 = sqrt(64)
NK = D // 128       # 8 k-tiles over the feature dim
NT = S // 128       # 32 j-tiles over keys
NI = BLK // 128     # 8 i-tiles over this core's queries
NJB = S // 512      # 8 key blocks of 512
E4 = ml_dtypes.float8_e4m3

_cache = {}


def _build_phase1():
    """G = x@M (fp16) and V' = x@W' (3-term fp8 DoubleRow) for this core's
    1024 rows.

    V' terms: x8hi@W8hi + x8lo@W8hi + x8hi@W8lo (lo*lo dropped, ~1.8e-3
    relative on V' which the softmax output inherits 1:1 -- inside budget).
    The host supplies the fp8 hi/lo splits of x^T and Wv@Wo.
    """
    import concourse.mybir as mybir
    from concourse import bacc
    from concourse.tile import TileContext

    FP16 = mybir.dt.float16
    FP32 = mybir.dt.float32
    FP8 = mybir.dt.float8e4
    DR = mybir.MatmulPerfMode.DoubleRow

    nc = bacc.Bacc("TRN2", target_bir_lowering=False, debug=False, num_devices=8)

    xt = nc.dram_tensor("xt", [D, BLK], FP16, kind="ExternalInput")  # rows.T
    wm = nc.dram_tensor("wm", [D, D], FP16, kind="ExternalInput")    # Wq@Wk^T
    xt8h = nc.dram_tensor("xt8h", [D, BLK], FP8, kind="ExternalInput")
    xt8l = nc.dram_tensor("xt8l", [D, BLK], FP8, kind="ExternalInput")
    wv8h = nc.dram_tensor("wv8h", [D, D], FP8, kind="ExternalInput")  # Wv@Wo
    wv8l = nc.dram_tensor("wv8l", [D, D], FP8, kind="ExternalInput")
    gt = nc.dram_tensor("gt", [D, BLK], FP16, kind="ExternalOutput")
    vo = nc.dram_tensor("vo", [NI, 128, D], FP16, kind="ExternalOutput")

    with TileContext(nc) as tc:
      with (
          tc.tile_pool(name="xp", bufs=1) as xp,
          tc.tile_pool(name="wp", bufs=1) as wp,
          tc.tile_pool(name="gps", bufs=4, space="PSUM") as gps,
          tc.tile_pool(name="gst", bufs=1) as gstp,
          tc.tile_pool(name="vps", bufs=2, space="PSUM") as vps,
          tc.tile_pool(name="wup", bufs=1, space="PSUM") as wup,
          tc.tile_pool(name="vsb", bufs=3) as vsbp,
          tc.tile_pool(name="ztp", bufs=1) as ztp,
      ):
        # short PE p-state warmup during the DMA lead-in (pe_busy_start is
        # pinned by the first matmul; clock is full 3us later)
        zt = ztp.tile([128, 128], FP16, name="zt", tag="zt")
        nc.gpsimd.memset(zt, 0.0)
        wt = wup.tile([128, 512], FP32, name="wu", tag="wu")
        for w in range(8):
            wsl = slice((w % 4) * 128, (w % 4 + 1) * 128)
            nc.tensor.matmul(wt[:, wsl], lhsT=zt, rhs=zt,
                             start=True, stop=True)
        # all loads on the SP queue in consumption order: the shared DMA
        # bus serializes transfers, so a single queue in priority order
        # beats spreading (racing queues invert priorities)
        xt_r = xt[:, :].rearrange("(n p) s -> p n s", p=128)
        x_sb = xp.tile([128, NK, BLK], FP16, name="x_sb", tag="x_sb")
        wm_sb = wp.tile([128, NK, D], FP16, name="wm_sb", tag="wm_sb")
        wm_r = wm[:, :].rearrange("(n p) d -> p n d", p=128)
        nc.sync.dma_start(x_sb[:, :, 0:128], xt_r[:, :, 0:128])
        nc.sync.dma_start(wm_sb[:, :, 0:128], wm_r[:, :, 0:128])
        nc.sync.dma_start(wm_sb[:, :, 128:256], wm_r[:, :, 128:256])
        nc.sync.dma_start(x_sb[:, :, 128:256], xt_r[:, :, 128:256])
        nc.sync.dma_start(wm_sb[:, :, 256:512], wm_r[:, :, 256:512])
        nc.sync.dma_start(x_sb[:, :, 256:512], xt_r[:, :, 256:512])
        nc.sync.dma_start(wm_sb[:, :, 512:D], wm_r[:, :, 512:D])
        nc.sync.dma_start(x_sb[:, :, 512:BLK], xt_r[:, :, 512:BLK])
        x8h_sb = xp.tile([128, NK, BLK], FP8, name="x8h_sb", tag="x8h_sb")
        x8l_sb = xp.tile([128, NK, BLK], FP8, name="x8l_sb", tag="x8l_sb")
        w8h_sb = wp.tile([128, NK, D], FP8, name="w8h_sb", tag="w8h_sb")
        w8l_sb = wp.tile([128, NK, D], FP8, name="w8l_sb", tag="w8l_sb")
        xt8h_r = xt8h[:, :].rearrange("(n p) s -> p n s", p=128)
        xt8l_r = xt8l[:, :].rearrange("(n p) s -> p n s", p=128)
        wv8h_r = wv8h[:, :].rearrange("(n p) d -> p n d", p=128)
        wv8l_r = wv8l[:, :].rearrange("(n p) d -> p n d", p=128)
        nc.sync.dma_start(x8h_sb, xt8h_r)
        nc.sync.dma_start(w8h_sb, wv8h_r)
        nc.sync.dma_start(x8l_sb, xt8l_r)
        nc.sync.dma_start(w8l_sb, wv8l_r)

        # G blocks emitted in DMA-supply order: each group becomes runnable
        # as one more of the loads above lands.
        # (x-chunk, wm m-block) emission order follows the load order above:
        # each group becomes runnable as one more load lands
        K_ORDER = [
            (0, 0), (0, 1),
            (1, 0), (1, 1),
            (0, 2), (0, 3), (1, 2), (1, 3),
            (2, 0), (2, 1), (2, 2), (2, 3),
            (0, 4), (0, 5), (1, 4), (1, 5), (2, 4), (2, 5),
            (0, 6), (0, 7), (1, 6), (1, 7), (2, 6), (2, 7),
            (3, 0), (3, 1), (3, 2), (3, 3), (3, 4), (3, 5), (3, 6), (3, 7),
        ]
        NBS = ((0, 128), (128, 128), (256, 256), (512, 512))
        st = []
        for m in range(NK):
            st.append(gstp.tile([128, BLK], FP16, name=f"gs{m}", tag=f"s{m}"))
        for (nb, m) in K_ORDER:
            n0, nw = NBS[nb]
            nsl = slice(n0, n0 + nw)
            msl = slice(m * 128, (m + 1) * 128)
            ps = gps.tile([128, 512], FP32, name=f"gp{n0}_{m}", tag="ps")
            for k in range(NK):
                nc.tensor.matmul(ps[:, 0:nw], lhsT=wm_sb[:, k, msl],
                                 rhs=x_sb[:, k, nsl],
                                 start=(k == 0), stop=(k == NK - 1))
            nc.vector.tensor_copy(st[m][:, nsl], ps[:, 0:nw])
        for m in range(NK):
            nc.sync.dma_start(gt[m * 128:(m + 1) * 128, :], st[m])

        V_TERMS = ((x8h_sb, w8h_sb), (x8l_sb, w8h_sb), (x8h_sb, w8l_sb))
        for j in range(NI):
            jsl = slice(j * 128, (j + 1) * 128)
            vt = vsbp.tile([128, D], FP16, name=f"vt{j}", tag="vt")
            last_j = j == NI - 1
            for db in range(2):
                dsl = slice(db * 512, (db + 1) * 512)
                ps = vps.tile([128, 512], FP32, name=f"vps{j}_{db}", tag="vps")
                for ti, (xa, wb) in enumerate(V_TERMS):
                    for kp in range(NK // 2):
                        ksl = slice(2 * kp, 2 * kp + 2)
                        nc.tensor.matmul(
                            ps, lhsT=xa[:, ksl, jsl], rhs=wb[:, ksl, dsl],
                            start=(ti == 0 and kp == 0),
                            stop=(ti == 2 and kp == NK // 2 - 1),
                            perf_mode=DR,
                        )
                if last_j and db == 1:
                    # split the final copy+store across queues to shorten
                    # the end-of-phase DMA drain chain
                    nc.vector.tensor_copy(vt[:, 512:768], ps[:, 0:256])
                    nc.scalar.dma_start(vo[j][:, 512:768], vt[:, 512:768])
                    nc.vector.tensor_copy(vt[:, 768:D], ps[:, 256:512])
                    nc.sync.dma_start(vo[j][:, 768:D], vt[:, 768:D])
                else:
                    nc.vector.tensor_copy(vt[:, dsl], ps)
                    nc.scalar.dma_start(vo[j][:, dsl], vt[:, dsl])
    nc.compile()
    return nc


def _build_phase2():
    """softmax(G_blk @ x^T * 8) @ V' for this core's 1024 queries.

    E/softmax layout as the fp16 version; P@V' in fp8 DoubleRow with a
    host-split V8hi/V8lo pair; P^T via DMA xbar transpose + GPSIMD cast.
    """
    import concourse.mybir as mybir
    from concourse import bacc
    from concourse.tile import TileContext

    FP16 = mybir.dt.float16
    FP32 = mybir.dt.float32
    FP8 = mybir.dt.float8e4
    Exp = mybir.ActivationFunctionType.Exp
    Copy = mybir.ActivationFunctionType.Copy
    AX = mybir.AxisListType.X
    DR = mybir.MatmulPerfMode.DoubleRow

    nc = bacc.Bacc("TRN2", target_bir_lowering=False, debug=False, num_devices=8)

    xth = nc.dram_tensor("xth", [D, S], FP16, kind="ExternalInput")
    # per-i-tile partition-major G: [i, p, n, f] = gt[n*128+p, i*128+f]
    gt2 = nc.dram_tensor("gt2", [NI, 128, NK, 128], FP16, kind="ExternalInput")
    # partition-major V' hi/lo fp8 pair: [p, t, d] = V'[t*128+p, d]
    v8h = nc.dram_tensor("v8h", [128, NT, D], FP8, kind="ExternalInput")
    v8l = nc.dram_tensor("v8l", [128, NT, D], FP8, kind="ExternalInput")
    y = nc.dram_tensor("y", [BLK, D], FP16, kind="ExternalOutput")

    from contextlib import ExitStack
    with TileContext(nc) as tc:
        with ExitStack() as stack:
            ztp = stack.enter_context(tc.tile_pool(name="ztp", bufs=1))
            wup = stack.enter_context(tc.tile_pool(name="wup", bufs=1,
                                                   space="PSUM"))
            ktp = stack.enter_context(tc.tile_pool(name="ktp", bufs=1))
            qtp = stack.enter_context(tc.tile_pool(name="qtp", bufs=4))
            vvp = stack.enter_context(tc.tile_pool(name="vvp", bufs=1))
            epsp = stack.enter_context(tc.tile_pool(name="eps", bufs=3, space="PSUM"))
            opsp = stack.enter_context(tc.tile_pool(name="ops", bufs=2, space="PSUM"))
            smp = stack.enter_context(tc.tile_pool(name="smp", bufs=2))
            esp = stack.enter_context(tc.tile_pool(name="esp", bufs=4))
            pp = stack.enter_context(tc.tile_pool(name="pp", bufs=3))
            ptp16 = stack.enter_context(tc.tile_pool(name="ptp16", bufs=2))
            ptp8 = stack.enter_context(tc.tile_pool(name="ptp8", bufs=2))
            obp = stack.enter_context(tc.tile_pool(name="obp", bufs=2))

            # PE p-state warmup during the DMA lead-in: pe_busy_start is
            # pinned by the FIRST matmul and survives small gaps, so a
            # short burst suffices (the clock is at 2.4GHz 3us later when
            # the first E matmul enters); rotating 4 psum slices avoids
            # back-to-back WAW sem stalls
            zt = ztp.tile([128, 128], FP16, name="zt", tag="zt")
            nc.gpsimd.memset(zt, 0.0)
            wt = wup.tile([128, 512], FP32, name="wu", tag="wu")
            for w in range(8):
                sl = slice((w % 4) * 128, (w % 4 + 1) * 128)
                nc.tensor.matmul(wt[:, sl], lhsT=zt, rhs=zt,
                                 start=True, stop=True)

            gv_t = [None] * NI

            def gv(i):
                return gv_t[i]

            gv_t[0] = qtp.tile([128, NK, 128], FP16, name="gv0", tag="gv")
            nc.sync.dma_start(gv_t[0], gt2[0])
            # x^T as one [128, NK, S] tile: each column chunk is a single
            # batched DMA covering all 8 k-rows; everything rides SP in
            # consumption order
            xth_r = xth[:, :].rearrange("(n p) s -> p n s", p=128)
            xth_all = ktp.tile([128, NK, S], FP16, name="xth_all", tag="xth")
            xth_sb = [xth_all[:, m, :] for m in range(NK)]
            vv8h = vvp.tile([128, NT, D], FP8, name="vv8h", tag="vv8h")
            vv8l = vvp.tile([128, NT, D], FP8, name="vv8l", tag="vv8l")
            nc.sync.dma_start(xth_all[:, :, 0:512], xth_r[:, :, 0:512])
            gv_t[1] = qtp.tile([128, NK, 128], FP16, name="gv1", tag="gv")
            nc.sync.dma_start(gv_t[1], gt2[1])
            for c0 in range(512, S, 512):
                nc.sync.dma_start(xth_all[:, :, c0:c0 + 512],
                                  xth_r[:, :, c0:c0 + 512])
            for i in range(2, 4):
                gv_t[i] = qtp.tile([128, NK, 128], FP16, name=f"gv{i}",
                                   tag="gv")
                nc.sync.dma_start(gv_t[i], gt2[i])
            nc.sync.dma_start(vv8h[:, 0:16, :], v8h[:, 0:16, :])
            nc.sync.dma_start(vv8h[:, 16:NT, :], v8h[:, 16:NT, :])
            nc.sync.dma_start(vv8l[:, 0:16, :], v8l[:, 0:16, :])
            nc.sync.dma_start(vv8l[:, 16:NT, :], v8l[:, 16:NT, :])

            st_mx8 = [None, None]
            eq_t = [[None, None], [None, None]]   # [i%2][half]

            def e_block(i, jb):
                sl = slice(jb * 512, (jb + 1) * 512)
                ps = epsp.tile([128, 512], FP32, name=f"eps{i}_{jb}", tag="eps")
                for k in range(NK):
                    nc.tensor.matmul(ps, lhsT=gv(i)[:, k, :],
                                     rhs=xth_sb[k][:, sl],
                                     start=(k == 0), stop=(k == NK - 1))
                half = jb % 4
                # psum->SBUF staging copy on DVE: keeps the ACT queue free
                # for the exp chain (a copy stuck behind 8 exps blocks this
                # psum buffer's reuse and stalls the PE)
                nc.vector.tensor_copy(
                    eq_t[i % 2][jb // 4][:, half * 512:(half + 1) * 512], ps)
                nc.vector.reduce_max(st_mx8[i % 2][:, jb:jb + 1], ps, axis=AX)

            def softmax_issue(i, rev=False):
                """Global max + exp chain (DVE stats + ACT exps) for i.

                rev=True (last i-tile) runs the half-1 exps first so its
                transpose+cast chain starts ~3us earlier -- the PV sweep for
                the last tile consumes half 1 first (see pv_sweep rev).
                """
                mx8 = st_mx8[i % 2]
                mrow = smp.tile([128, 1], FP32, name=f"mrow{i}", tag="mrow")
                nc.vector.reduce_max(mrow, mx8, axis=AX)
                negm = smp.tile([128, 1], FP32, name=f"negm{i}", tag="negm")
                nc.vector.tensor_scalar_mul(negm, mrow, -SCALE)
                p_h = [pp.tile([128, S // 2], FP16, name=f"p{i}_{h}", tag="p")
                       for h in range(2)]
                lp8 = smp.tile([128, NJB], FP32, name=f"lp8_{i}", tag="lp8")
                jbs = list(range(NJB))
                if rev:
                    jbs = jbs[4:] + jbs[:4]
                for jb in jbs:
                    half = jb % 4
                    nc.scalar.activation(
                        p_h[jb // 4][:, (jb % 4) * 512:(jb % 4) * 512 + 512],
                        eq_t[i % 2][jb // 4][:, half * 512:(half + 1) * 512],
                        Exp, bias=negm, scale=SCALE,
                        accum_out=lp8[:, jb:jb + 1],
                    )
                lrow = smp.tile([128, 1], FP32, name=f"lrow{i}", tag="lrow")
                nc.vector.reduce_sum(lrow, lp8, axis=AX)
                linv = smp.tile([128, 1], FP32, name=f"linv{i}", tag="linv")
                nc.vector.reciprocal(linv, lrow)
                return p_h, linv

            def pt_issue(i, p_h, split=False):
                """P^T via DMA xbar transpose (SP queue) + GPSIMD fp8 cast.

                pt16[jp, t, q] = p_h[h][q, t*128+jp]; global key tile
                index is h*16+t, so pt8[:, t', :] holds P^T for key tile
                t' in exactly the DoubleRow lhsT layout.  split=True (used
                for the last i-tile, where nothing else queues behind it)
                halves the cast latency by running Pool and DVE in
                parallel on each transposed half.
                """
                pt8 = ptp8.tile([128, NT, 128], FP8, name=f"pt8_{i}",
                                tag="pt8")
                for h in ((1, 0) if split else (0, 1)):
                    pt16 = ptp16.tile([128, 16, 128], FP16,
                                      name=f"pt16_{i}_{h}", tag="pt16")
                    nc.sync.dma_start(pt16, p_h[h], transpose=True)
                    if split:
                        nc.gpsimd.tensor_copy(
                            pt8[:, h * 16:h * 16 + 8, :], pt16[:, 0:8, :])
                        nc.vector.tensor_copy(
                            pt8[:, h * 16 + 8:h * 16 + 16, :], pt16[:, 8:16, :])
                    else:
                        nc.gpsimd.tensor_copy(pt8[:, h * 16:(h + 1) * 16, :],
                                              pt16)
                return pt8

            def pv_sweep(i, pt8, linv, rev=False):
                """P@V' as two fp8 DoubleRow sweeps (V8hi then V8lo) into
                one PSUM accumulation group per output half."""
                op0 = opsp.tile([128, 512], FP32, name=f"op0_{i}", tag="op0")
                op1 = opsp.tile([128, 512], FP32, name=f"op1_{i}", tag="op1")
                osb = obp.tile([128, D], FP16, name=f"osb{i}", tag="osb")
                # key-half-major order: the t<8 pair sweeps only read the
                # first 2048 keys of pt8, so the PE can start as soon as the
                # first half's transpose+cast lands while half 1 is still in
                # flight (rev: half 1 first, matching the last tile's
                # reversed exp/cast order)
                hs = (1, 0) if rev else (0, 1)
                for hi_, h in enumerate(hs):
                    for term, vv in ((0, vv8h), (1, vv8l)):
                        for tt in range(NT // 4):
                            t = h * (NT // 4) + tt
                            sl = slice(2 * t, 2 * t + 2)
                            first = hi_ == 0 and term == 0 and tt == 0
                            last = (hi_ == 1 and term == 1
                                    and tt == NT // 4 - 1)
                            nc.tensor.matmul(op0, lhsT=pt8[:, sl, :],
                                             rhs=vv[:, sl, 0:512],
                                             start=first, stop=last,
                                             perf_mode=DR)
                            nc.tensor.matmul(op1, lhsT=pt8[:, sl, :],
                                             rhs=vv[:, sl, 512:D],
                                             start=first, stop=last,
                                             perf_mode=DR)
                # store halves on separate queues as each scale lands (the
                # last tile's drain chain shortens by ~1us)
                nc.scalar.activation(osb[:, 0:512], op0, Copy, scale=linv)
                nc.sync.dma_start(y[i * 128:(i + 1) * 128, 0:512],
                                  osb[:, 0:512])
                nc.scalar.activation(osb[:, 512:D], op1, Copy, scale=linv)
                nc.scalar.dma_start(y[i * 128:(i + 1) * 128, 512:D],
                                    osb[:, 512:D])

            sm = [None] * NI
            pt8_t = [None] * NI

            def E_tile(i, jbs):
                if jbs[0] == 0:
                    st_mx8[i % 2] = smp.tile([128, NJB], FP32, name=f"mx8_{i}",
                                             tag=f"mx8{i % 2}")
                for jb in jbs:
                    if jb % 4 == 0:
                        eq_t[i % 2][jb // 4] = esp.tile(
                            [128, 2048], FP32, name=f"e{i}_{jb // 4}", tag="e")
                    e_block(i, jb)

            # head: E(0)/E(1) interleaved jb-major over the x^T chunk
            # supply; E(0) finishes first so exp(0) starts early
            for jb in range(6):
                E_tile(0, [jb])
                E_tile(1, [jb])
            E_tile(0, [6, 7])
            sm[0] = softmax_issue(0)
            pt8_t[0] = pt_issue(0, sm[0][0])
            E_tile(1, [6, 7])
            # lag-2 pipeline: E(i); exps(i-1) (after E(i)'s psum copies so
            # the ACT queue drains in dependency order); sweep(i-2)
            for i in range(2, NI):
                E_tile(i, list(range(NJB)))
                if i + 2 < NI:
                    gv_t[i + 2] = qtp.tile([128, NK, 128], FP16,
                                           name=f"gv{i + 2}", tag="gv")
                    nc.sync.dma_start(gv_t[i + 2], gt2[i + 2])
                sm[i - 1] = softmax_issue(i - 1)
                pt8_t[i - 1] = pt_issue(i - 1, sm[i - 1][0])
                pv_sweep(i - 2, pt8_t[i - 2], sm[i - 2][1])
            sm[NI - 1] = softmax_issue(NI - 1, rev=True)
            pt8_t[NI - 1] = pt_issue(NI - 1, sm[NI - 1][0], split=True)
            pv_sweep(NI - 2, pt8_t[NI - 2], sm[NI - 2][1])
            pv_sweep(NI - 1, pt8_t[NI - 1], sm[NI - 1][1], rev=True)
    nc.compile()
    return nc


def _get_programs():
    if "nc1" not in _cache:
        _cache["nc1"] = _build_phase1()
        _cache["nc2"] = _build_phase2()
    return _cache["nc1"], _cache["nc2"]


def kernel(x, Wq, Wk, Wv, Wo):
    from concourse.bass_utils import run_bass_kernel_spmd

    nc1, nc2 = _get_programs()

    x = np.asarray(x, dtype=np.float32)
    # fold the weights once on the host (associativity):
    #   energy = x (Wq Wk^T) x^T ;  out = attn (x (Wv Wo))
    wm = (np.asarray(Wq, np.float64) @ np.asarray(Wk, np.float64).T
          ).astype(np.float16)
    wvo = (np.asarray(Wv, np.float64) @ np.asarray(Wo, np.float64)
           ).astype(np.float32)
    # scale Wv@Wo (std ~0.03) up into e4m3's normal range before the hi/lo
    # split -- unscaled it sits entirely in the subnormal region and the
    # split loses ~1.8e-2 relative; the device then computes 32*V' and the
    # host divides it back out of vo below
    wvs = wvo * 32.0
    wv8h = wvs.astype(E4)
    wv8l = (wvs - wv8h.astype(np.float32)).astype(E4)

    # ---- phase 1: per-core row slices ----
    in1 = []
    for c in range(8):
        b, i = divmod(c, 4)
        rows = x[b, i * BLK:(i + 1) * BLK, :]           # [BLK, D]
        rt = np.ascontiguousarray(rows.T)               # [D, BLK] fp32
        x8h = rt.astype(E4)
        x8l = (rt - x8h.astype(np.float32)).astype(E4)
        in1.append({
            "xt": rt.astype(np.float16),
            "xt8h": x8h, "xt8l": x8l,
            "wm": wm, "wv8h": wv8h, "wv8l": wv8l,
        })
    res1 = run_bass_kernel_spmd(nc1, in1, list(range(8))).results

    # ---- host gather of V' shards; fp8 hi/lo split; x^T cast per batch ----
    xth_full, v8h_full, v8l_full = [], [], []
    for b in range(B):
        xth_full.append(np.ascontiguousarray(
            x[b].T.astype(np.float16)))                  # [D, S]
        v = np.concatenate(
            [res1[b * 4 + i]["vo"] for i in range(4)], axis=0)    # [NT, 128, D]
        vp = np.ascontiguousarray(v.transpose(1, 0, 2)).astype(np.float32)
        vp *= 1.0 / 32.0          # undo the Wv@Wo fp8-range prescale
        vh = vp.astype(E4)
        vl = (vp - vh.astype(np.float32)).astype(E4)
        v8h_full.append(vh)                              # [128, NT, D] fp8
        v8l_full.append(vl)

    # ---- phase 2 ----
    in2 = []
    for c in range(8):
        b, i = divmod(c, 4)
        gstack = res1[c]["gt"].reshape(NK, 128, NI, 128)  # [n, p, i, f]
        in2.append({
            "xth": xth_full[b], "v8h": v8h_full[b], "v8l": v8l_full[b],
            "gt2": np.ascontiguousarray(gstack.transpose(2, 1, 0, 3)),
        })
    res2 = run_bass_kernel_spmd(nc2, in2, list(range(8))).results

    out = np.empty((B, S, D), dtype=np.float32)
    for c in range(8):
        b, i = divmod(c, 4)
        out[b, i * BLK:(i + 1) * BLK, :] = res2[c]["y"].astype(np.float32)
    return out


# revision 27
# speedup vs baseline: 1.0414x; 1.0414x over previous
"""Trainium2 Bass kernel for single-head self-attention (B=2, S=4096, D=1024).

reference:
    q = x @ Wq; k = x @ Wk; v = x @ Wv          # [B,S,D]
    energy = einsum('bid,bjd->bij', q, k) * 8.0  # SCALE = sqrt(64)
    attn = softmax(energy, axis=-1)
    out = einsum('bij,bjd->bid', attn, v) @ Wo

Weight folding (associativity): energy = x @ (Wq Wk^T) @ x^T and
out = attn @ (x @ (Wv Wo)), so the host precomputes M = Wq @ Wk^T and
W' = Wv @ Wo once (fp64) and the device only runs TWO projections
(G = x@M, V' = x@W') plus the two S^2-sized attention matmuls.

Two SPMD launches over 8 cores (= 2 batches x 4 query-blocks of 1024):
  phase 1: each core computes G / V' for its own 1024 rows (fp16).
  phase 2: each core computes softmax(G_blk @ x^T * 8) @ V' for its
           1024 queries against the full batch.

Phase-2 speed tricks (vs the plain fp16 version):
  * P @ V' runs in fp8-e4m3 DoubleRow mode (K=256 per instruction,
    0.5 PE-cycles/row): the host splits V' into V8hi + V8lo (hi/lo fp8
    pair, residual ~7.5e-4 relative), P is a single fp8 (softmax rows
    are ~99.2% one-hot; quantizing the off-argmax tail costs ~2e-3).
    Two DR sweeps (hi then lo) accumulate into the same PSUM group, so
    the P@V' matmul costs 2*16 instead of 64 fp16-equivalents per
    i-tile: 131072 PE-cycles total instead of 262144.
  * P^T is produced by the DMA xbar transpose (14ns per 16x128 tile on
    the DMA queue; zero PE cycles) into fp16, then GPSIMD (otherwise
    idle) convert-copies it to the fp8 DR operand. This removes all
    256 PE transposes and their DVE psum->SBUF copies.
  * The logit path (G = x@M and E = G@x^T) stays fp16: logit std is
    ~300 (SCALE multiplies) and fp8 quantization noise there flips
    near-tie argmaxes.  fp16 keeps logit noise ~0.13 -> ~8e-3 rel err.
  * PE p-state warmup matmuls run during the DMA lead-in (HAM reaches
    2.4GHz after ~3us of sustained PE activity).

Queue assignment: bulk loads ride SP in consumption order; the ACT
queue handles E psum->SBUF copies + the softmax exp chain + output
scales; DVE takes the reduces; SP also carries the P DMA-transposes
and y stores (issued in dependency order); GPSIMD does the fp8 casts.
"""

import numpy as np
import ml_dtypes

B, S, D = 2, 4096, 1024
BLK = 1024          # queries per core
SCALE = 8.0         # HEAD_DIM ** 0.5 = sqrt(64)
NK = D // 128       # 8 k-tiles over the feature dim
NT = S // 128       # 32 j-tiles over keys
NI = BLK // 128     # 8 i-tiles over this core's queries
NJB = S // 512      # 8 key blocks of 512
E4 = ml_dtypes.float8_e4m3

_cache = {}


def _build_phase1():
    """G = x@M (fp16) and V' = x@W' (3-term fp8 DoubleRow) for this core's
    1024 rows.

    V' terms: x8hi@W8hi + x8lo@W8hi + x8hi@W8lo (lo*lo dropped, ~1.8e-3
    relative on V' which the softmax output inherits 1:1 -- inside budget).
    The host supplies the fp8 hi/lo splits of x^T and Wv@Wo.
    """
    import concourse.mybir as mybir
    from concourse import bacc
    from concourse.tile import TileContext

    FP16 = mybir.dt.float16
    FP32 = mybir.dt.float32
    FP8 = mybir.dt.float8e4
    DR = mybir.MatmulPerfMode.DoubleRow

    nc = bacc.Bacc("TRN2", target_bir_lowering=False, debug=False, num_devices=8)

    xt = nc.dram_tensor("xt", [D, BLK], FP16, kind="ExternalInput")  # rows.T
    wm = nc.dram_tensor("wm", [D, D], FP16, kind="ExternalInput")    # Wq@Wk^T
    xt8h = nc.dram_tensor("xt8h", [D, BLK], FP8, kind="ExternalInput")
    xt8l = nc.dram_tensor("xt8l", [D, BLK], FP8, kind="ExternalInput")
    wv8h = nc.dram_tensor("wv8h", [D, D], FP8, kind="ExternalInput")  # Wv@Wo
    wv8l = nc.dram_tensor("wv8l", [D, D], FP8, kind="ExternalInput")
    gt = nc.dram_tensor("gt", [D, BLK], FP16, kind="ExternalOutput")
    vo = nc.dram_tensor("vo", [NI, 128, D], FP16, kind="ExternalOutput")

    with TileContext(nc) as tc:
      with (
          tc.tile_pool(name="xp", bufs=1) as xp,
          tc.tile_pool(name="wp", bufs=1) as wp,
          tc.tile_pool(name="gps", bufs=4, space="PSUM") as gps,
          tc.tile_pool(name="gst", bufs=1) as gstp,
          tc.tile_pool(name="vps", bufs=2, space="PSUM") as vps,
          tc.tile_pool(name="wup", bufs=1, space="PSUM") as wup,
          tc.tile_pool(name="vsb", bufs=3) as vsbp,
          tc.tile_pool(name="ztp", bufs=1) as ztp,
      ):
        # short PE p-state warmup during the DMA lead-in (pe_busy_start is
        # pinned by the first matmul; clock is full 3us later)
        zt = ztp.tile([128, 128], FP16, name="zt", tag="zt")
        nc.gpsimd.memset(zt, 0.0)
        wt = wup.tile([128, 512], FP32, name="wu", tag="wu")
        for w in range(8):
            wsl = slice((w % 4) * 128, (w % 4 + 1) * 128)
            nc.tensor.matmul(wt[:, wsl], lhsT=zt, rhs=zt,
                             start=True, stop=True)
        # all loads on the SP queue in consumption order: the shared DMA
        # bus serializes transfers, so a single queue in priority order
        # beats spreading (racing queues invert priorities)
        xt_r = xt[:, :].rearrange("(n p) s -> p n s", p=128)
        x_sb = xp.tile([128, NK, BLK], FP16, name="x_sb", tag="x_sb")
        wm_sb = wp.tile([128, NK, D], FP16, name="wm_sb", tag="wm_sb")
        wm_r = wm[:, :].rearrange("(n p) d -> p n d", p=128)
        nc.sync.dma_start(x_sb[:, :, 0:128], xt_r[:, :, 0:128])
        nc.sync.dma_start(wm_sb[:, :, 0:128], wm_r[:, :, 0:128])
        nc.sync.dma_start(wm_sb[:, :, 128:256], wm_r[:, :, 128:256])
        nc.sync.dma_start(x_sb[:, :, 128:256], xt_r[:, :, 128:256])
        nc.sync.dma_start(wm_sb[:, :, 256:512], wm_r[:, :, 256:512])
        nc.sync.dma_start(x_sb[:, :, 256:512], xt_r[:, :, 256:512])
        nc.sync.dma_start(wm_sb[:, :, 512:D], wm_r[:, :, 512:D])
        nc.sync.dma_start(x_sb[:, :, 512:BLK], xt_r[:, :, 512:BLK])
        x8h_sb = xp.tile([128, NK, BLK], FP8, name="x8h_sb", tag="x8h_sb")
        x8l_sb = xp.tile([128, NK, BLK], FP8, name="x8l_sb", tag="x8l_sb")
        w8h_sb = wp.tile([128, NK, D], FP8, name="w8h_sb", tag="w8h_sb")
        w8l_sb = wp.tile([128, NK, D], FP8, name="w8l_sb", tag="w8l_sb")
        xt8h_r = xt8h[:, :].rearrange("(n p) s -> p n s", p=128)
        xt8l_r = xt8l[:, :].rearrange("(n p) s -> p n s", p=128)
        wv8h_r = wv8h[:, :].rearrange("(n p) d -> p n d", p=128)
        wv8l_r = wv8l[:, :].rearrange("(n p) d -> p n d", p=128)
        nc.sync.dma_start(x8h_sb, xt8h_r)
        nc.sync.dma_start(w8h_sb, wv8h_r)
        nc.sync.dma_start(x8l_sb, xt8l_r)
        nc.sync.dma_start(w8l_sb, wv8l_r)

        # G blocks emitted in DMA-supply order: each group becomes runnable
        # as one more of the loads above lands.
        # (x-chunk, wm m-block) emission order follows the load order above:
        # each group becomes runnable as one more load lands
        K_ORDER = [
            (0, 0), (0, 1),
            (1, 0), (1, 1),
            (0, 2), (0, 3), (1, 2), (1, 3),
            (2, 0), (2, 1), (2, 2), (2, 3),
            (0, 4), (0, 5), (1, 4), (1, 5), (2, 4), (2, 5),
            (0, 6), (0, 7), (1, 6), (1, 7), (2, 6), (2, 7),
            (3, 0), (3, 1), (3, 2), (3, 3), (3, 4), (3, 5), (3, 6), (3, 7),
        ]
        NBS = ((0, 128), (128, 128), (256, 256), (512, 512))
        st = []
        for m in range(NK):
            st.append(gstp.tile([128, BLK], FP16, name=f"gs{m}", tag=f"s{m}"))
        for (nb, m) in K_ORDER:
            n0, nw = NBS[nb]
            nsl = slice(n0, n0 + nw)
            msl = slice(m * 128, (m + 1) * 128)
            ps = gps.tile([128, 512], FP32, name=f"gp{n0}_{m}", tag="ps")
            for k in range(NK):
                nc.tensor.matmul(ps[:, 0:nw], lhsT=wm_sb[:, k, msl],
                                 rhs=x_sb[:, k, nsl],
                                 start=(k == 0), stop=(k == NK - 1))
            nc.vector.tensor_copy(st[m][:, nsl], ps[:, 0:nw])
        for m in range(NK):
            nc.sync.dma_start(gt[m * 128:(m + 1) * 128, :], st[m])

        V_TERMS = ((x8h_sb, w8h_sb), (x8l_sb, w8h_sb), (x8h_sb, w8l_sb))
        for j in range(NI):
            jsl = slice(j * 128, (j + 1) * 128)
            vt = vsbp.tile([128, D], FP16, name=f"vt{j}", tag="vt")
            last_j = j == NI - 1
            for db in range(2):
                dsl = slice(db * 512, (db + 1) * 512)
                ps = vps.tile([128, 512], FP32, name=f"vps{j}_{db}", tag="vps")
                for ti, (xa, wb) in enumerate(V_TERMS):
                    for kp in range(NK // 2):
                        ksl = slice(2 * kp, 2 * kp + 2)
                        nc.tensor.matmul(
                            ps, lhsT=xa[:, ksl, jsl], rhs=wb[:, ksl, dsl],
                            start=(ti == 0 and kp == 0),
                            stop=(ti == 2 and kp == NK // 2 - 1),
                            perf_mode=DR,
                        )
                if last_j and db == 1:
                    # split the final copy+store across queues to shorten
                    # the end-of-phase DMA drain chain
                    nc.vector.tensor_copy(vt[:, 512:768], ps[:, 0:256])
                    nc.scalar.dma_start(vo[j][:, 512:768], vt[:, 512:768])
                    nc.vector.tensor_copy(vt[:, 768:D], ps[:, 256:512])
                    nc.sync.dma_start(vo[j][:, 768:D], vt[:, 768:D])
                else:
                    nc.vector.tensor_copy(vt[:, dsl], ps)
                    nc.scalar.dma_start(vo[j][:, dsl], vt[:, dsl])
    nc.compile()
    return nc


def _build_phase2():
    """softmax(G_blk @ x^T * 8) @ V' for this core's 1024 queries.

    E/softmax layout as the fp16 version; P@V' in fp8 DoubleRow with a
    host-split V8hi/V8lo pair; P^T via DMA xbar transpose + GPSIMD cast.
    """
    import concourse.mybir as mybir
    from concourse import bacc
    from concourse.tile import TileContext

    FP16 = mybir.dt.float16
    FP32 = mybir.dt.float32
    FP8 = mybir.dt.float8e4
    Exp = mybir.ActivationFunctionType.Exp
    Copy = mybir.ActivationFunctionType.Copy
    AX = mybir.AxisListType.X
    DR = mybir.MatmulPerfMode.DoubleRow

    nc = bacc.Bacc("TRN2", target_bir_lowering=False, debug=False, num_devices=8)

    xth = nc.dram_tensor("xth", [D, S], FP16, kind="ExternalInput")
    # per-i-tile partition-major G: [i, p, n, f] = gt[n*128+p, i*128+f]
    gt2 = nc.dram_tensor("gt2", [NI, 128, NK, 128], FP16, kind="ExternalInput")
    # partition-major V' hi/lo fp8 pair: [p, t, d] = V'[t*128+p, d]
    v8h = nc.dram_tensor("v8h", [128, NT, D], FP8, kind="ExternalInput")
    v8l = nc.dram_tensor("v8l", [128, NT, D], FP8, kind="ExternalInput")
    y = nc.dram_tensor("y", [BLK, D], FP16, kind="ExternalOutput")

    from contextlib import ExitStack
    with TileContext(nc) as tc:
        with ExitStack() as stack:
            ztp = stack.enter_context(tc.tile_pool(name="ztp", bufs=1))
            wup = stack.enter_context(tc.tile_pool(name="wup", bufs=1,
                                                   space="PSUM"))
            ktp = stack.enter_context(tc.tile_pool(name="ktp", bufs=1))
            qtp = stack.enter_context(tc.tile_pool(name="qtp", bufs=4))
            vvp = stack.enter_context(tc.tile_pool(name="vvp", bufs=1))
            epsp = stack.enter_context(tc.tile_pool(name="eps", bufs=3, space="PSUM"))
            opsp = stack.enter_context(tc.tile_pool(name="ops", bufs=2, space="PSUM"))
            smp = stack.enter_context(tc.tile_pool(name="smp", bufs=2))
            esp = stack.enter_context(tc.tile_pool(name="esp", bufs=4))
            pp = stack.enter_context(tc.tile_pool(name="pp", bufs=3))
            ptp16 = stack.enter_context(tc.tile_pool(name="ptp16", bufs=2))
            ptp8 = stack.enter_context(tc.tile_pool(name="ptp8", bufs=2))
            obp = stack.enter_context(tc.tile_pool(name="obp", bufs=2))

            # PE p-state warmup during the DMA lead-in: pe_busy_start is
            # pinned by the FIRST matmul and survives small gaps, so a
            # short burst suffices (the clock is at 2.4GHz 3us later when
            # the first E matmul enters); rotating 4 psum slices avoids
            # back-to-back WAW sem stalls
            zt = ztp.tile([128, 128], FP16, name="zt", tag="zt")
            nc.gpsimd.memset(zt, 0.0)
            wt = wup.tile([128, 512], FP32, name="wu", tag="wu")
            for w in range(8):
                sl = slice((w % 4) * 128, (w % 4 + 1) * 128)
                nc.tensor.matmul(wt[:, sl], lhsT=zt, rhs=zt,
                                 start=True, stop=True)

            gv_t = [None] * NI

            def gv(i):
                return gv_t[i]

            gv_t[0] = qtp.tile([128, NK, 128], FP16, name="gv0", tag="gv")
            nc.sync.dma_start(gv_t[0], gt2[0])
            # x^T as one [128, NK, S] tile: each column chunk is a single
            # batched DMA covering all 8 k-rows; everything rides SP in
            # consumption order
            xth_r = xth[:, :].rearrange("(n p) s -> p n s", p=128)
            xth_all = ktp.tile([128, NK, S], FP16, name="xth_all", tag="xth")
            xth_sb = [xth_all[:, m, :] for m in range(NK)]
            vv8h = vvp.tile([128, NT, D], FP8, name="vv8h", tag="vv8h")
            vv8l = vvp.tile([128, NT, D], FP8, name="vv8l", tag="vv8l")
            nc.sync.dma_start(xth_all[:, :, 0:512], xth_r[:, :, 0:512])
            gv_t[1] = qtp.tile([128, NK, 128], FP16, name="gv1", tag="gv")
            nc.sync.dma_start(gv_t[1], gt2[1])
            for c0 in range(512, S, 512):
                nc.sync.dma_start(xth_all[:, :, c0:c0 + 512],
                                  xth_r[:, :, c0:c0 + 512])
            for i in range(2, 4):
                gv_t[i] = qtp.tile([128, NK, 128], FP16, name=f"gv{i}",
                                   tag="gv")
                nc.sync.dma_start(gv_t[i], gt2[i])
            nc.sync.dma_start(vv8h[:, 0:16, :], v8h[:, 0:16, :])
            nc.sync.dma_start(vv8h[:, 16:NT, :], v8h[:, 16:NT, :])
            nc.sync.dma_start(vv8l[:, 0:16, :], v8l[:, 0:16, :])
            nc.sync.dma_start(vv8l[:, 16:NT, :], v8l[:, 16:NT, :])

            st_mx8 = [None, None]
            eq_t = [[None, None], [None, None]]   # [i%2][half]

            def e_block(i, jb):
                sl = slice(jb * 512, (jb + 1) * 512)
                ps = epsp.tile([128, 512], FP32, name=f"eps{i}_{jb}", tag="eps")
                for k in range(NK):
                    nc.tensor.matmul(ps, lhsT=gv(i)[:, k, :],
                                     rhs=xth_sb[k][:, sl],
                                     start=(k == 0), stop=(k == NK - 1))
                half = jb % 4
                # psum->SBUF staging copy on DVE: keeps the ACT queue free
                # for the exp chain (a copy stuck behind 8 exps blocks this
                # psum buffer's reuse and stalls the PE)
                nc.vector.tensor_copy(
                    eq_t[i % 2][jb // 4][:, half * 512:(half + 1) * 512], ps)
                nc.vector.reduce_max(st_mx8[i % 2][:, jb:jb + 1], ps, axis=AX)

            def softmax_issue(i, rev=False):
                """Global max + exp chain (DVE stats + ACT exps) for i.

                rev=True (last i-tile) runs the half-1 exps first so its
                transpose+cast chain starts ~3us earlier -- the PV sweep for
                the last tile consumes half 1 first (see pv_sweep rev).
                """
                mx8 = st_mx8[i % 2]
                mrow = smp.tile([128, 1], FP32, name=f"mrow{i}", tag="mrow")
                nc.vector.reduce_max(mrow, mx8, axis=AX)
                negm = smp.tile([128, 1], FP32, name=f"negm{i}", tag="negm")
                nc.vector.tensor_scalar_mul(negm, mrow, -SCALE)
                p_h = [pp.tile([128, S // 2], FP16, name=f"p{i}_{h}", tag="p")
                       for h in range(2)]
                lp8 = smp.tile([128, NJB], FP32, name=f"lp8_{i}", tag="lp8")
                jbs = list(range(NJB))
                if rev:
                    jbs = jbs[4:] + jbs[:4]
                for jb in jbs:
                    half = jb % 4
                    nc.scalar.activation(
                        p_h[jb // 4][:, (jb % 4) * 512:(jb % 4) * 512 + 512],
                        eq_t[i % 2][jb // 4][:, half * 512:(half + 1) * 512],
                        Exp, bias=negm, scale=SCALE,
                        accum_out=lp8[:, jb:jb + 1],
                    )
                lrow = smp.tile([128, 1], FP32, name=f"lrow{i}", tag="lrow")
                nc.vector.reduce_sum(lrow, lp8, axis=AX)
                linv = smp.tile([128, 1], FP32, name=f"linv{i}", tag="linv")
                nc.vector.reciprocal(linv, lrow)
                return p_h, linv

            def pt_issue(i, p_h, split=False):
                """P^T via DMA xbar transpose (SP queue) + GPSIMD fp8 cast.

                pt16[jp, t, q] = p_h[h][q, t*128+jp]; global key tile
                index is h*16+t, so pt8[:, t', :] holds P^T for key tile
                t' in exactly the DoubleRow lhsT layout.  split=True (used
                for the last i-tile, where nothing else queues behind it)
                halves the cast latency by running Pool and DVE in
                parallel on each transposed half.
                """
                pt8 = ptp8.tile([128, NT, 128], FP8, name=f"pt8_{i}",
                                tag="pt8")
                for h in ((1, 0) if split else (0, 1)):
                    pt16 = ptp16.tile([128, 16, 128], FP16,
                                      name=f"pt16_{i}_{h}", tag="pt16")
                    nc.sync.dma_start(pt16, p_h[h], transpose=True)
                    if split:
                        nc.gpsimd.tensor_copy(
                            pt8[:, h * 16:h * 16 + 8, :], pt16[:, 0:8, :])
                        nc.vector.tensor_copy(
                            pt8[:, h * 16 + 8:h * 16 + 16, :], pt16[:, 8:16, :])
                    else:
                        nc.gpsimd.tensor_copy(pt8[:, h * 16:(h + 1) * 16, :],
                                              pt16)
                return pt8

            def pv_sweep(i, pt8, linv, rev=False):
                """P@V' as two fp8 DoubleRow sweeps (V8hi then V8lo) into
                one PSUM accumulation group per output half."""
                op0 = opsp.tile([128, 512], FP32, name=f"op0_{i}", tag="op0")
                op1 = opsp.tile([128, 512], FP32, name=f"op1_{i}", tag="op1")
                osb = obp.tile([128, D], FP16, name=f"osb{i}", tag="osb")
                # key-half-major order: the t<8 pair sweeps only read the
                # first 2048 keys of pt8, so the PE can start as soon as the
                # first half's transpose+cast lands while half 1 is still in
                # flight (rev: half 1 first, matching the last tile's
                # reversed exp/cast order)
                hs = (1, 0) if rev else (0, 1)
                for hi_, h in enumerate(hs):
                    for term, vv in ((0, vv8h), (1, vv8l)):
                        for tt in range(NT // 4):
                            t = h * (NT // 4) + tt
                            sl = slice(2 * t, 2 * t + 2)
                            first = hi_ == 0 and term == 0 and tt == 0
                            last = (hi_ == 1 and term == 1
                                    and tt == NT // 4 - 1)
                            nc.tensor.matmul(op0, lhsT=pt8[:, sl, :],
                                             rhs=vv[:, sl, 0:512],
                                             start=first, stop=last,
                                             perf_mode=DR)
                            nc.tensor.matmul(op1, lhsT=pt8[:, sl, :],
                                             rhs=vv[:, sl, 512:D],
                                             start=first, stop=last,
                                             perf_mode=DR)
                nc.scalar.activation(osb[:, 0:512], op0, Copy, scale=linv)
                nc.scalar.activation(osb[:, 512:D], op1, Copy, scale=linv)
                nc.sync.dma_start(y[i * 128:(i + 1) * 128, :], osb)

            sm = [None] * NI
            pt8_t = [None] * NI

            def E_tile(i, jbs):
                if jbs[0] == 0:
                    st_mx8[i % 2] = smp.tile([128, NJB], FP32, name=f"mx8_{i}",
                                             tag=f"mx8{i % 2}")
                for jb in jbs:
                    if jb % 4 == 0:
                        eq_t[i % 2][jb // 4] = esp.tile(
                            [128, 2048], FP32, name=f"e{i}_{jb // 4}", tag="e")
                    e_block(i, jb)

            # head: E(0)/E(1) interleaved jb-major over the x^T chunk
            # supply; E(0) finishes first so exp(0) starts early
            for jb in range(6):
                E_tile(0, [jb])
                E_tile(1, [jb])
            E_tile(0, [6, 7])
            sm[0] = softmax_issue(0)
            pt8_t[0] = pt_issue(0, sm[0][0])
            E_tile(1, [6, 7])
            # lag-2 pipeline: E(i); exps(i-1) (after E(i)'s psum copies so
            # the ACT queue drains in dependency order); sweep(i-2)
            for i in range(2, NI):
                E_tile(i, list(range(NJB)))
                if i + 2 < NI:
                    gv_t[i + 2] = qtp.tile([128, NK, 128], FP16,
                                           name=f"gv{i + 2}", tag="gv")
                    nc.sync.dma_start(gv_t[i + 2], gt2[i + 2])
                sm[i - 1] = softmax_issue(i - 1)
                pt8_t[i - 1] = pt_issue(i - 1, sm[i - 1][0])
                pv_sweep(i - 2, pt8_t[i - 2], sm[i - 2][1])
            sm[NI - 1] = softmax_issue(NI - 1, rev=True)
            pt8_t[NI - 1] = pt_issue(NI - 1, sm[NI - 1][0], split=True)
            pv_sweep(NI - 2, pt8_t[NI - 2], sm[NI - 2][1])
            pv_sweep(NI - 1, pt8_t[NI - 1], sm[NI - 1][1], rev=True)
    nc.compile()
    return nc


def _get_programs():
    if "nc1" not in _cache:
        _cache["nc1"] = _build_phase1()
        _cache["nc2"] = _build_phase2()
    return _cache["nc1"], _cache["nc2"]


def kernel(x, Wq, Wk, Wv, Wo):
    from concourse.bass_utils import run_bass_kernel_spmd

    nc1, nc2 = _get_programs()

    x = np.asarray(x, dtype=np.float32)
    # fold the weights once on the host (associativity):
    #   energy = x (Wq Wk^T) x^T ;  out = attn (x (Wv Wo))
    wm = (np.asarray(Wq, np.float64) @ np.asarray(Wk, np.float64).T
          ).astype(np.float16)
    wvo = (np.asarray(Wv, np.float64) @ np.asarray(Wo, np.float64)
           ).astype(np.float32)
    # scale Wv@Wo (std ~0.03) up into e4m3's normal range before the hi/lo
    # split -- unscaled it sits entirely in the subnormal region and the
    # split loses ~1.8e-2 relative; the device then computes 32*V' and the
    # host divides it back out of vo below
    wvs = wvo * 32.0
    wv8h = wvs.astype(E4)
    wv8l = (wvs - wv8h.astype(np.float32)).astype(E4)

    # ---- phase 1: per-core row slices ----
    in1 = []
    for c in range(8):
        b, i = divmod(c, 4)
        rows = x[b, i * BLK:(i + 1) * BLK, :]           # [BLK, D]
        rt = np.ascontiguousarray(rows.T)               # [D, BLK] fp32
        x8h = rt.astype(E4)
        x8l = (rt - x8h.astype(np.float32)).astype(E4)
        in1.append({
            "xt": rt.astype(np.float16),
            "xt8h": x8h, "xt8l": x8l,
            "wm": wm, "wv8h": wv8h, "wv8l": wv8l,
        })
    res1 = run_bass_kernel_spmd(nc1, in1, list(range(8))).results

    # ---- host gather of V' shards; fp8 hi/lo split; x^T cast per batch ----
    xth_full, v8h_full, v8l_full = [], [], []
    for b in range(B):
        xth_full.append(np.ascontiguousarray(
            x[b].T.astype(np.float16)))                  # [D, S]
        v = np.concatenate(
            [res1[b * 4 + i]["vo"] for i in range(4)], axis=0)    # [NT, 128, D]
        vp = np.ascontiguousarray(v.transpose(1, 0, 2)).astype(np.float32)
        vp *= 1.0 / 32.0          # undo the Wv@Wo fp8-range prescale
        vh = vp.astype(E4)
        vl = (vp - vh.astype(np.float32)).astype(E4)
        v8h_full.append(vh)                              # [128, NT, D] fp8
        v8l_full.append(vl)

    # ---- phase 2 ----
    in2 = []
    for c in range(8):
        b, i = divmod(c, 4)
        gstack = res1[c]["gt"].reshape(NK, 128, NI, 128)  # [n, p, i, f]
        in2.append({
            "xth": xth_full[b], "v8h": v8h_full[b], "v8l": v8l_full[b],
            "gt2": np.ascontiguousarray(gstack.transpose(2, 1, 0, 3)),
        })
    res2 = run_bass_kernel_spmd(nc2, in2, list(range(8))).results

    out = np.empty((B, S, D), dtype=np.float32)
    for c in range(8):
        b, i = divmod(c, 4)
        out[b, i * BLK:(i + 1) * BLK, :] = res2[c]["y"].astype(np.float32)
    return out
